# revision 1
# baseline (speedup 1.0000x reference)
"""Trainium2 Bass kernel for nn_KANSplineLayer (KAN spline layer, 8-core SPMD).

Math rewrite (validated to 6e-7 L2 rel err vs reference in fp32):
  reference: out = silu(BN_b(x @ Wb)) + BN_s(basis(minmax(x)) @ Ws.T)
  with 9 wide triangle-basis functions per input feature.

  Because each per-(o,i) spline g(z) = sum_k w[o,i,k]*tri_k(z) is continuous
  piecewise-linear on z in [0,1] with breakpoints {0,.25,.5,.75,1}, it equals
  a linear combination of {t, relu(t-1), relu(t-2), relu(t-3), 1} with
  t = 4*z in [0,4).  This shrinks the spline GEMM contraction from
  256*9=2304 to 256*4=1024 (+1 bias rank-1 term) and turns the basis
  construction into 1-op-per-plane elementwise work.

Sharding: data-parallel over rows (batch*H*W = 32768 -> 4096 rows/core).
Global per-feature min/max via a [128,4] AllReduce(min) on (min, -max).

Device pipeline per core:
  phase 1: DMA x tiles -> DVE stage -> PE transpose -> x^T in SBUF
           + DVE min/max reduction over rows
  collective: AllReduce(min) of [min | -max]
  phase 2: planes t = (x^T - min)*s4, r_m = relu(t - m)  (DVE/ACT)
           GEMMs (fp32r, full PE rate) into PSUM [rows, spline|base]
           epilogue: silu(base half) + spline half -> out rows
All PE matmul/transpose operands are produced by DVE so each PE
instruction needs at most one semaphore wait (walrus S3_LW limit).
"""
import numpy as np

import concourse.bacc as bacc
import concourse.bass as bass
import concourse.tile as tile
from concourse import mybir
from concourse.bass_utils import run_bass_kernel_spmd

# ---- problem constants (hardcoded; kernel.py must be self-contained) ----
IN_F, OUT_F = 256, 256
K_KNOTS = 9
EPS_MINMAX = 1e-7
EPS_BN = 1e-3
B, H, W = 32, 32, 32
N_TOTAL = B * H * W            # 32768 rows
N_CORES = 8
N_SHARD = N_TOTAL // N_CORES   # 4096 rows per core
R_TILES = N_SHARD // 128       # 32 row tiles per core
CH = 512                       # phase-2 column chunk (rows of output)
N_CHUNKS = N_SHARD // CH

F32 = mybir.dt.float32
MM_DT = mybir.dt.float32r      # full-rate fp32 matmul mode (N>=256)


def _host_prep(base_weight, spline_weight, spline_scaler,
               bn_base_gamma, bn_base_beta, bn_base_mean, bn_base_var,
               bn_spline_gamma, bn_spline_beta, bn_spline_mean, bn_spline_var):
    """Fold BN + rewrite spline into relu-plane weights. All in float64."""
    f64 = np.float64
    w = np.asarray(spline_weight, f64) * np.asarray(spline_scaler, f64)[:, :, None]
    knots = np.linspace(-1.0, 1.0, K_KNOTS).astype(f64)
    jg = np.arange(5, dtype=f64) / 4.0
    tri = np.maximum(0.0, 1.0 - np.abs(jg[None, :] - knots[:, None]))   # [k, j]
    G = np.einsum('oik,kj->oij', w, tri)                                # [o,i,5]
    a_s = np.asarray(bn_spline_gamma, f64) / np.sqrt(np.asarray(bn_spline_var, f64) + EPS_BN)
    b_s = np.asarray(bn_spline_beta, f64) - a_s * np.asarray(bn_spline_mean, f64)
    G = G * a_s[:, None, None]
    W_t = (G[:, :, 1] - G[:, :, 0]).T                                   # [i,o]
    H1 = (G[:, :, 2] - 2 * G[:, :, 1] + G[:, :, 0]).T
    H2 = (G[:, :, 3] - 2 * G[:, :, 2] + G[:, :, 1]).T
    H3 = (G[:, :, 4] - 2 * G[:, :, 3] + G[:, :, 2]).T
    C_s = G[:, :, 0].sum(axis=1) + b_s                                  # [o]
    a_b = np.asarray(bn_base_gamma, f64) / np.sqrt(np.asarray(bn_base_var, f64) + EPS_BN)
    b_b = np.asarray(bn_base_beta, f64) - a_b * np.asarray(bn_base_mean, f64)
    Wb = np.asarray(base_weight, f64) * a_b[None, :]                    # [i,o]
    f32 = np.float32
    w_t = np.stack([W_t[b * 128:(b + 1) * 128] for b in range(2)]).astype(f32)
    w_base = np.stack([Wb[b * 128:(b + 1) * 128] for b in range(2)]).astype(f32)
    w_r = np.stack([
        np.stack([Hm[b * 128:(b + 1) * 128] for b in range(2)])
        for Hm in (H1, H2, H3)]).astype(f32)                            # [3,2,128,256]
    bias_row = np.concatenate([C_s, b_b]).astype(f32)[None, :]          # [1,512]
    return w_t, w_base, w_r, bias_row


def _build_bass():
    nc = bacc.Bacc(num_devices=N_CORES)
    x_sh = nc.declare_dram_parameter("x_sh", [N_SHARD, IN_F], F32, isOutput=False)
    w_t_d = nc.declare_dram_parameter("w_t", [2, 128, 256], F32, isOutput=False)
    w_b_d = nc.declare_dram_parameter("w_base", [2, 128, 256], F32, isOutput=False)
    w_r_d = nc.declare_dram_parameter("w_r", [3, 2, 128, 256], F32, isOutput=False)
    bias_d = nc.declare_dram_parameter("bias_row", [1, 512], F32, isOutput=False)
    ident_d = nc.declare_dram_parameter("ident", [128, 128], F32, isOutput=False)
    out_sh = nc.declare_dram_parameter("out_sh", [N_SHARD, OUT_F], F32, isOutput=True)

    from contextlib import ExitStack
    with tile.TileContext(nc) as tc, ExitStack() as es:
        cons = es.enter_context(tc.tile_pool(name="cons", bufs=1))
        stage = es.enter_context(tc.tile_pool(name="stage", bufs=3))
        xin_p = es.enter_context(tc.tile_pool(name="xin", bufs=3))
        psT = es.enter_context(tc.tile_pool(name="psT", bufs=4, space="PSUM"))
        psM = es.enter_context(tc.tile_pool(name="psM", bufs=4, space="PSUM"))
        planes_p = es.enter_context(tc.tile_pool(name="planes", bufs=2))
        outp = es.enter_context(tc.tile_pool(name="outp", bufs=4))
        dram = es.enter_context(tc.tile_pool(name="dram", bufs=2, space="DRAM"))
        if True:
            # ---- constants, staged through DVE so PE waits stay single-sem ----
            def dve_load(nm, shape, dram_ap, dt=MM_DT):
                tmp = stage.tile(shape, F32, tag="ldtmp", name=f"ld_{nm}")
                nc.sync.dma_start(out=tmp[:], in_=dram_ap)
                t = cons.tile(shape, dt, tag=nm, name=nm)
                nc.vector.tensor_copy(out=t[:], in_=tmp[:])
                return t

            ident = dve_load("ident", [128, 128], ident_d[:], dt=F32)
            wt_sb = dve_load("wt_sb", [128, 2, 256], w_t_d.rearrange("b p n -> p b n"))
            wb_sb = dve_load("wb_sb", [128, 2, 256], w_b_d.rearrange("b p n -> p b n"))
            wr = dve_load("wr", [128, 3, 2, 256], w_r_d.rearrange("m b p n -> p m b n"))
            bias_sb = dve_load("bias_sb", [1, 512], bias_d[:])
            ones_f32 = cons.tile([1, 128], F32)
            nc.vector.memset(ones_f32[:], 1.0)
            ones = cons.tile([1, 128], MM_DT)
            nc.vector.tensor_copy(out=ones[:], in_=ones_f32[:])
            rb = cons.tile([128, 2], F32)     # ACT Relu biases -1, -2
            nc.vector.memset(rb[:, 0:1], -1.0)
            nc.vector.memset(rb[:, 1:2], -2.0)

            # x^T, feature blocks on partitions; fp32r so it can feed base GEMMs
            xt = cons.tile([128, 2, N_SHARD], MM_DT)

            # ---- phase 1: load + transpose + local min/max ----
            for r in range(R_TILES):
                xin = xin_p.tile([128, IN_F], F32)
                nc.sync.dma_start(out=xin[:], in_=x_sh[r * 128:(r + 1) * 128, :])
                xst = stage.tile([128, IN_F], F32, tag="xst")
                nc.vector.tensor_copy(out=xst[:], in_=xin[:])
                for b in range(2):
                    pst = psT.tile([128, 128], F32)
                    nc.tensor.transpose(pst[:], xst[:, b * 128:(b + 1) * 128], ident[:])
                    nc.vector.tensor_copy(
                        out=xt[:, b, r * 128:(r + 1) * 128], in_=pst[:])

            mm_loc = cons.tile([128, 4], F32)   # [min0, min1, -max0, -max1]
            lmax = cons.tile([128, 2], F32)
            for b in range(2):
                nc.vector.tensor_reduce(
                    out=mm_loc[:, b:b + 1], in_=xt[:, b, :],
                    op=mybir.AluOpType.min, axis=mybir.AxisListType.X)
                nc.vector.tensor_reduce(
                    out=lmax[:, b:b + 1], in_=xt[:, b, :],
                    op=mybir.AluOpType.max, axis=mybir.AxisListType.X)
            nc.vector.tensor_scalar(
                out=mm_loc[:, 2:4], in0=lmax[:], scalar1=-1.0, scalar2=None,
                op0=mybir.AluOpType.mult)

            # ---- global min/max across the 8 cores ----
            cc_in = dram.tile([128, 4], F32)
            cc_out = dram.tile([128, 4], F32)
            nc.sync.dma_start(out=cc_in[:], in_=mm_loc[:])
            nc.gpsimd.collective_compute(
                "AllReduce", mybir.AluOpType.min,
                replica_groups=[list(range(N_CORES))],
                ins=[cc_in.opt()], outs=[cc_out.opt()])
            gmm = cons.tile([128, 4], F32)       # [gmin0, gmin1, -gmax0, -gmax1]
            nc.sync.dma_start(out=gmm[:], in_=cc_out[:])

            # s4 = 4/(gmax-gmin+eps); t = (x - gmin)*s4
            nrng = cons.tile([128, 2], F32)
            qt = cons.tile([128, 2], F32)
            s4 = cons.tile([128, 2], F32)
            for b in range(2):
                nc.vector.tensor_tensor(
                    out=nrng[:, b:b + 1], in0=gmm[:, b:b + 1],
                    in1=gmm[:, 2 + b:3 + b], op=mybir.AluOpType.add)  # gmin-gmax
            nc.vector.tensor_scalar(
                out=qt[:], in0=nrng[:], scalar1=-0.25, scalar2=EPS_MINMAX * 0.25,
                op0=mybir.AluOpType.mult, op1=mybir.AluOpType.add)
            nc.vector.reciprocal(out=s4[:], in_=qt[:])

            # ---- phase 2: planes + GEMMs + epilogue ----
            for c in range(N_CHUNKS):
                cs = slice(c * CH, (c + 1) * CH)
                tpl = [planes_p.tile([128, CH], MM_DT, tag=f"t{b}", name=f"t{b}_{c}")
                       for b in range(2)]
                rpl = [[planes_p.tile([128, CH], MM_DT, tag=f"r{m}{b}", name=f"r{m}{b}_{c}")
                        for b in range(2)] for m in range(3)]
                for b in range(2):
                    # t = (x^T - gmin) * s4   (DVE, per-partition scalars)
                    nc.vector.tensor_scalar(
                        out=tpl[b][:], in0=xt[:, b, cs],
                        scalar1=gmm[:, b:b + 1], scalar2=s4[:, b:b + 1],
                        op0=mybir.AluOpType.subtract, op1=mybir.AluOpType.mult)
                    # r1/r2 on ACT, r3 on DVE
                    for m in (1, 2):
                        nc.scalar.activation(
                            out=rpl[m - 1][b][:], in_=tpl[b][:],
                            func=mybir.ActivationFunctionType.Relu,
                            bias=rb[:, m - 1:m], scale=1.0)
                    nc.vector.tensor_scalar(
                        out=rpl[2][b][:], in0=tpl[b][:], scalar1=3.0, scalar2=0.0,
                        op0=mybir.AluOpType.subtract, op1=mybir.AluOpType.max)
                for j in range(CH // 128):
                    js = slice(j * 128, (j + 1) * 128)
                    ps = psM.tile([128, 512], F32)
                    # rank-1 bias: ones^T @ [C_s | b_b]
                    nc.tensor.matmul(
                        ps[:], ones[:], bias_sb[:],
                        start=True, stop=False, skip_group_check=True)
                    for b in range(2):
                        nc.tensor.matmul(
                            ps[:, 0:256], tpl[b][:, js], wt_sb[:, b, :],
                            start=False, stop=False, skip_group_check=True)
                        nc.tensor.matmul(
                            ps[:, 256:512], xt[:, b, c * CH + j * 128:c * CH + (j + 1) * 128],
                            wb_sb[:, b, :],
                            start=False, stop=False, skip_group_check=True)
                    for m in range(3):
                        for b in range(2):
                            nc.tensor.matmul(
                                ps[:, 0:256], rpl[m][b][:, js], wr[:, m, b, :],
                                start=False, stop=(m == 2 and b == 1),
                                skip_group_check=True)
                    o = outp.tile([128, OUT_F], F32)
                    nc.scalar.activation(
                        out=o[:], in_=ps[:, 256:512],
                        func=mybir.ActivationFunctionType.Silu)
                    nc.vector.tensor_tensor(
                        out=o[:], in0=o[:], in1=ps[:, 0:256],
                        op=mybir.AluOpType.add)
                    r0 = c * CH + j * 128
                    nc.sync.dma_start(out=out_sh[r0:r0 + 128, :], in_=o[:])
    nc.compile()
    return nc


_CACHE = {}


def make_in_maps(inputs):
    x = np.ascontiguousarray(np.asarray(inputs["x"], np.float32))
    w_t, w_base, w_r, bias_row = _host_prep(
        **{k: v for k, v in inputs.items() if k != "x"})
    ident = np.eye(128, dtype=np.float32)
    xf = x.reshape(N_TOTAL, IN_F)
    return [{
        "x_sh": np.ascontiguousarray(xf[c * N_SHARD:(c + 1) * N_SHARD]),
        "w_t": w_t, "w_base": w_base, "w_r": w_r, "bias_row": bias_row,
        "ident": ident,
    } for c in range(N_CORES)]


def kernel(**inputs):
    if "nc" not in _CACHE:
        _CACHE["nc"] = _build_bass()
    nc = _CACHE["nc"]
    in_maps = make_in_maps(inputs)
    res = run_bass_kernel_spmd(nc, in_maps, list(range(N_CORES)))
    out = np.concatenate([res.results[c]["out_sh"] for c in range(N_CORES)], axis=0)
    return out.reshape(B, H, W, OUT_F).astype(np.float32)



# revision 5
# speedup vs baseline: 1.3193x; 1.3193x over previous
"""Trainium2 Bass kernel for nn_KANSplineLayer (KAN spline layer, 8-core SPMD).

Math rewrite (same as v0, validated): the 9-triangle spline per (o,i) is a
continuous piecewise-linear function of t = 4*minmax(x) in [0,4], so it equals
a combination of {t, relu(t-1), relu(t-2), relu(t-3), 1}.  Spline contraction
shrinks from 256*9 to 256*4 (+rank-1 bias).

v1 restructure (everything in fp16 on device, PSUM accum fp32):
  - host casts x to fp16; x^T loaded with DMA XBAR transpose straight from
    DRAM (no PE transposes, no stage copies).
  - min/max partials per DMA segment; AllReduce(min) of [min|-max] fired
    ~60us earlier than v0.
  - output kept transposed ([outs, rows]) so BN/silu biases are per-partition
    (free via ACT bias); base path GEMM+silu runs during the collective.
  - weights stationary in PE, planes/x moving with N=512 => 1 cyc/row fp16.
  - fp16 output (host re-transposes + casts), halves output DMA bytes.
"""
import numpy as np

import concourse.bacc as bacc
import concourse.bass as bass
import concourse.tile as tile
from concourse import mybir
from concourse.bass_utils import run_bass_kernel_spmd

# ---- problem constants (hardcoded; kernel.py must be self-contained) ----
IN_F, OUT_F = 256, 256
K_KNOTS = 9
EPS_MINMAX = 1e-7
EPS_BN = 1e-3
B, H, W = 32, 32, 32
N_TOTAL = B * H * W            # 32768 rows
N_CORES = 8
N_SHARD = N_TOTAL // N_CORES   # 4096 rows per core
CH = 512                       # phase-2 row chunk (moving dim)
N_CHUNKS = N_SHARD // CH       # 8
N_SEG = 4                      # x^T DMA-transpose segments (per feature block: 2)
SEG = N_SHARD // 2             # 2048 rows per (block, half) segment

F32 = mybir.dt.float32
F16 = mybir.dt.float16


def _host_prep(base_weight, spline_weight, spline_scaler,
               bn_base_gamma, bn_base_beta, bn_base_mean, bn_base_var,
               bn_spline_gamma, bn_spline_beta, bn_spline_mean, bn_spline_var):
    """Fold BN + rewrite spline into relu-plane weights. All in float64.

    Returns SBUF-layout arrays:
      w_t  [128, 2, 2, 128]   (i', b, bo, o')  t-plane weights
      w_r  [128, 3, 2, 2, 128] (i', m, b, bo, o') relu-plane weights
      w_b  [128, 2, 2, 128]   base weights
      cs   [1, 256]           spline constant (bo-major)
      bb   [128, 2]           base bias per (o', bo), f32
    """
    f64 = np.float64
    w = np.asarray(spline_weight, f64) * np.asarray(spline_scaler, f64)[:, :, None]
    knots = np.linspace(-1.0, 1.0, K_KNOTS).astype(f64)
    jg = np.arange(5, dtype=f64) / 4.0
    tri = np.maximum(0.0, 1.0 - np.abs(jg[None, :] - knots[:, None]))   # [k, j]
    G = np.einsum('oik,kj->oij', w, tri)                                # [o,i,5]
    a_s = np.asarray(bn_spline_gamma, f64) / np.sqrt(np.asarray(bn_spline_var, f64) + EPS_BN)
    b_s = np.asarray(bn_spline_beta, f64) - a_s * np.asarray(bn_spline_mean, f64)
    G = G * a_s[:, None, None]
    W_t = (G[:, :, 1] - G[:, :, 0]).T                                   # [i,o]
    Hs = [(G[:, :, 2] - 2 * G[:, :, 1] + G[:, :, 0]).T,
          (G[:, :, 3] - 2 * G[:, :, 2] + G[:, :, 1]).T,
          (G[:, :, 4] - 2 * G[:, :, 3] + G[:, :, 2]).T]                 # [i,o]
    C_s = G[:, :, 0].sum(axis=1) + b_s                                  # [o]
    a_b = np.asarray(bn_base_gamma, f64) / np.sqrt(np.asarray(bn_base_var, f64) + EPS_BN)
    b_b = np.asarray(bn_base_beta, f64) - a_b * np.asarray(bn_base_mean, f64)
    Wb = np.asarray(base_weight, f64) * a_b[None, :]                    # [i,o]

    def blk(M):  # [in, out] f64 -> [128, 2(b), 2(bo), 128] (i', b, bo, o')
        return M.reshape(2, 128, 2, 128).transpose(1, 0, 2, 3)

    w_t = blk(W_t).astype(np.float16)
    w_b = blk(Wb).astype(np.float16)
    w_r = np.stack([blk(Hm) for Hm in Hs], axis=1).astype(np.float16)  # [128,3,2,2,128]
    cs = C_s.astype(np.float16)[None, :]                               # [1, 256]
    bb = b_b.reshape(2, 128).T.astype(np.float32)                      # [128, 2]
    return w_t, w_r, w_b, cs, bb


def _build_bass(use_pool_epilogue=False):
    nc = bacc.Bacc(num_devices=N_CORES)
    x_sh = nc.declare_dram_parameter("x_sh", [N_SHARD, IN_F], F16, isOutput=False)
    w_t_d = nc.declare_dram_parameter("w_t", [128, 2, 2, 128], F16, isOutput=False)
    w_r_d = nc.declare_dram_parameter("w_r", [128, 3, 2, 2, 128], F16, isOutput=False)
    w_b_d = nc.declare_dram_parameter("w_b", [128, 2, 2, 128], F16, isOutput=False)
    cs_d = nc.declare_dram_parameter("cs", [1, 256], F16, isOutput=False)
    bb_d = nc.declare_dram_parameter("bb", [128, 2], F32, isOutput=False)
    out_t = nc.declare_dram_parameter("out_t", [2, 128, N_SHARD], F16, isOutput=True)

    from contextlib import ExitStack
    with tile.TileContext(nc) as tc, ExitStack() as es:
        cons = es.enter_context(tc.tile_pool(name="cons", bufs=1))
        silu_p = es.enter_context(tc.tile_pool(name="silu", bufs=1))
        planes_p = es.enter_context(tc.tile_pool(name="planes", bufs=2))
        psS = es.enter_context(tc.tile_pool(name="psS", bufs=4, space="PSUM"))
        psB = es.enter_context(tc.tile_pool(name="psB", bufs=2, space="PSUM"))
        outp = es.enter_context(tc.tile_pool(name="outp", bufs=3))
        dram = es.enter_context(tc.tile_pool(name="dram", bufs=2, space="DRAM"))

        # ---- weight loads (scalar/ACT queue; x transposes go on sync) ----
        wb_sb = cons.tile([128, 2, 2, 128], F16, name="wb_sb")
        nc.scalar.dma_start(out=wb_sb[:], in_=w_b_d[:])
        wt_sb = cons.tile([128, 2, 2, 128], F16, name="wt_sb")
        nc.scalar.dma_start(out=wt_sb[:], in_=w_t_d[:])
        wr_sb = cons.tile([128, 3, 2, 2, 128], F16, name="wr_sb")
        nc.scalar.dma_start(out=wr_sb[:], in_=w_r_d[:])
        cs_sb = cons.tile([1, 256], F16, name="cs_sb")
        nc.scalar.dma_start(out=cs_sb[:], in_=cs_d[:])
        bb_sb = cons.tile([128, 2], F32, name="bb_sb")
        nc.scalar.dma_start(out=bb_sb[:], in_=bb_d[:])

        ones_f32 = cons.tile([1, CH], F32, name="ones_f32")
        nc.vector.memset(ones_f32[:], 1.0)
        ones = cons.tile([1, CH], F16, name="ones")
        nc.vector.tensor_copy(out=ones[:], in_=ones_f32[:])
        rb = cons.tile([128, 2], F32, name="rb")     # ACT Relu biases -1, -2
        nc.vector.memset(rb[:, 0:1], -1.0)
        nc.vector.memset(rb[:, 1:2], -2.0)

        # ---- phase 1: x^T via DMA XBAR transpose, segmented minmax ----
        xt = cons.tile([128, 2, N_SHARD], F16, name="xt")
        for b in range(2):
            for h in range(2):
                nc.sync.dma_start(
                    out=xt[:, b, h * SEG:(h + 1) * SEG],
                    in_=x_sh[h * SEG:(h + 1) * SEG, b * 128:(b + 1) * 128],
                    transpose=True)

        pmin = cons.tile([128, 2, 2], F32, name="pmin")   # (b, h)
        pmax = cons.tile([128, 2, 2], F32, name="pmax")
        for b in range(2):
            for h in range(2):
                seg = xt[:, b, h * SEG:(h + 1) * SEG]
                nc.vector.tensor_reduce(
                    out=pmin[:, b, h:h + 1], in_=seg,
                    op=mybir.AluOpType.min, axis=mybir.AxisListType.X)
                nc.vector.tensor_reduce(
                    out=pmax[:, b, h:h + 1], in_=seg,
                    op=mybir.AluOpType.max, axis=mybir.AxisListType.X)
        mm_loc = cons.tile([128, 4], F32, name="mm_loc")  # [min0,min1,-max0,-max1]
        tmax = cons.tile([128, 2], F32, name="tmax")
        nc.vector.tensor_tensor(
            out=mm_loc[:, 0:2], in0=pmin[:, :, 0], in1=pmin[:, :, 1],
            op=mybir.AluOpType.min)
        nc.vector.tensor_tensor(
            out=tmax[:], in0=pmax[:, :, 0], in1=pmax[:, :, 1],
            op=mybir.AluOpType.max)
        nc.vector.tensor_scalar(
            out=mm_loc[:, 2:4], in0=tmax[:], scalar1=-1.0, scalar2=None,
            op0=mybir.AluOpType.mult)

        # ---- global min/max across the 8 cores ----
        cc_in = dram.tile([128, 4], F32)
        cc_out = dram.tile([128, 4], F32)
        nc.sync.dma_start(out=cc_in[:], in_=mm_loc[:])
        nc.gpsimd.collective_compute(
            "AllReduce", mybir.AluOpType.min,
            replica_groups=[list(range(N_CORES))],
            ins=[cc_in.opt()], outs=[cc_out.opt()])
        gmm = cons.tile([128, 4], F32, name="gmm")   # [gmin0,gmin1,-gmax0,-gmax1]
        nc.sync.dma_start(out=gmm[:], in_=cc_out[:])

        # s4 = 4/(gmax-gmin+eps)
        nrng = cons.tile([128, 2], F32, name="nrng")
        qt = cons.tile([128, 2], F32, name="qt")
        s4 = cons.tile([128, 2], F32, name="s4")
        nc.vector.tensor_tensor(
            out=nrng[:], in0=gmm[:, 0:2], in1=gmm[:, 2:4],
            op=mybir.AluOpType.add)                       # gmin - gmax
        nc.vector.tensor_scalar(
            out=qt[:], in0=nrng[:], scalar1=-0.25, scalar2=EPS_MINMAX * 0.25,
            op0=mybir.AluOpType.mult, op1=mybir.AluOpType.add)
        nc.vector.reciprocal(out=s4[:], in_=qt[:])

        # ---- phase 1.5: base path (x @ Wb, silu w/ per-partition bias) ----
        # runs on PE/ACT while the collective is in flight.
        silu_sb = [[silu_p.tile([128, CH], F16, name=f"silu_{c}_{bo}")
                    for bo in range(2)] for c in range(N_CHUNKS)]
        for c in range(N_CHUNKS):
            cs_sl = slice(c * CH, (c + 1) * CH)
            for bo in range(2):
                pb = psB.tile([128, CH], F32)
                for b in range(2):
                    nc.tensor.matmul(
                        pb[:], wb_sb[:, b, bo, :], xt[:, b, cs_sl],
                        start=(b == 0), stop=(b == 1), skip_group_check=True)
                nc.scalar.activation(
                    out=silu_sb[c][bo][:], in_=pb[:],
                    func=mybir.ActivationFunctionType.Silu,
                    bias=bb_sb[:, bo:bo + 1], scale=1.0)

        # ---- phase 2: planes + spline GEMMs + epilogue ----
        for c in range(N_CHUNKS):
            cs_sl = slice(c * CH, (c + 1) * CH)
            tpl = [planes_p.tile([128, CH], F16, tag=f"t{b}", name=f"t{b}_{c}")
                   for b in range(2)]
            rpl = [[planes_p.tile([128, CH], F16, tag=f"r{m}{b}", name=f"r{m}{b}_{c}")
                    for b in range(2)] for m in range(3)]
            for b in range(2):
                nc.vector.tensor_scalar(
                    out=tpl[b][:], in0=xt[:, b, cs_sl],
                    scalar1=gmm[:, b:b + 1], scalar2=s4[:, b:b + 1],
                    op0=mybir.AluOpType.subtract, op1=mybir.AluOpType.mult)
                for m in (1, 2):
                    nc.scalar.activation(
                        out=rpl[m - 1][b][:], in_=tpl[b][:],
                        func=mybir.ActivationFunctionType.Relu,
                        bias=rb[:, m - 1:m], scale=1.0)
                nc.vector.tensor_scalar(
                    out=rpl[2][b][:], in0=tpl[b][:], scalar1=3.0, scalar2=0.0,
                    op0=mybir.AluOpType.subtract, op1=mybir.AluOpType.max)
            for bo in range(2):
                ps = psS.tile([128, CH], F32)
                nc.tensor.matmul(
                    ps[:], cs_sb[:, bo * 128:(bo + 1) * 128], ones[:],
                    start=True, stop=False, skip_group_check=True)
                for b in range(2):
                    nc.tensor.matmul(
                        ps[:], wt_sb[:, b, bo, :], tpl[b][:],
                        start=False, stop=False, skip_group_check=True)
                for m in range(3):
                    for b in range(2):
                        nc.tensor.matmul(
                            ps[:], wr_sb[:, m, b, bo, :], rpl[m][b][:],
                            start=False, stop=(m == 2 and b == 1),
                            skip_group_check=True)
                o = outp.tile([128, CH], F16)
                eng = nc.gpsimd if use_pool_epilogue else nc.vector
                eng.tensor_tensor(
                    out=o[:], in0=ps[:], in1=silu_sb[c][bo][:],
                    op=mybir.AluOpType.add)
                nc.sync.dma_start(out=out_t[bo, :, cs_sl], in_=o[:])
    nc.compile()
    return nc


_CACHE = {}


def make_in_maps(inputs):
    x = np.asarray(inputs["x"], np.float32).reshape(N_TOTAL, IN_F)
    x16 = x.astype(np.float16)
    w_t, w_r, w_b, cs, bb = _host_prep(
        **{k: v for k, v in inputs.items() if k != "x"})
    return [{
        "x_sh": np.ascontiguousarray(x16[c * N_SHARD:(c + 1) * N_SHARD]),
        "w_t": w_t, "w_r": w_r, "w_b": w_b, "cs": cs, "bb": bb,
    } for c in range(N_CORES)]


def kernel(**inputs):
    if "nc" not in _CACHE:
        _CACHE["nc"] = _build_bass()
    nc = _CACHE["nc"]
    in_maps = make_in_maps(inputs)
    res = run_bass_kernel_spmd(nc, in_maps, list(range(N_CORES)))
    out = np.empty((N_TOTAL, OUT_F), np.float32)
    for c in range(N_CORES):
        ot = np.asarray(res.results[c]["out_t"], np.float32)  # [2,128,4096]
        out[c * N_SHARD:(c + 1) * N_SHARD] = (
            ot.transpose(2, 0, 1).reshape(N_SHARD, OUT_F))
    return out.reshape(B, H, W, OUT_F)
